# revision 83
# baseline (speedup 1.0000x reference)
"""Trainium2 Bass kernel for a pre-norm cross-attention transformer layer.

Reference computation (B=4, Lq=Lk=1024, E=1024, H=16, Dh=64, F=4096):
    t2 = LN(tgt); q = t2@wq+bq; k = mem@wk+bk; v = mem@wv+bv
    p = softmax(q k^T / sqrt(Dh)); attn = mean_h(p)
    x = tgt + (p v)@wo + bo
    out = x + relu(LN(x)@w1+b1)@w2 + b2
Returns (out, attn).

Sharding: 8 cores = 4 batches x 2 query-halves. Each core owns 512 query rows
of one batch, computes K/V for its batch's full memory (duplicated within the
pair), and produces disjoint slices of both outputs -> no collectives.

On-device layout: all activations are kept transposed ([features, rows]) so
every matmul's contraction dim sits on SBUF partitions. The host passes
pre-transposed inputs and un-transposes outputs.

Precision: the q/k/v/o projections, the scores matmul, and FFN1 run in fp8
e4m3 with power-of-2 static scales folded into the PSUM-eviction activations;
fp8 pairs of contraction k-tiles are fed to the PE in DoubleRow perf mode
(two ifmap rows per cycle). The scores matmul gets its DoubleRow pair from a
Pool-engine partition-shift copy of q/k into [32, 2, .] layout (the 64-dim
head contraction split in half). FFN2 stays bf16: its operands' quantization
would add ~1.9% output error on its own, while the whole attention side adds
only ~0.1% (residual-dominated outputs). The softmax probabilities stay bf16
(p@v, head-mean): fp8 operands run 1.65x slower on the DVE, which is the
attention-phase bottleneck. Accumulation is fp32 in PSUM; softmax/LN
bookkeeping is fp32. Measured end-to-end: out rel err 1.88e-2, attn 8.3e-3
(gate 2e-2).

Softmax denominators come for free: wv is host-augmented with one extra
all-zero column per head whose bias row is 1, so the p@v accumulation's 65th
output row is sum_k exp(score). 1/denom is exp(-ln d) on ACT (the plain DVE
reciprocal costs 3.3us/call; walrus cannot codegen the custom-op approx
variants), with the head-mean scale folded into the exp bias so the
mean-accumulation is a plain tensor_mul.
"""

import math
import os
import sys
from contextlib import ExitStack

for _p in ("/opt/trn_rl_repo", "/root/.axon_site/_ro/trn_rl_repo"):
    if os.path.isdir(_p) and _p not in sys.path:
        sys.path.append(_p)

import ml_dtypes
import numpy as np

import concourse.bass as bass
import concourse.tile as tile
from concourse import mybir
from concourse.bass_utils import run_bass_kernel_spmd
from concourse.vector_clock import ScopedClock
from concourse.tile import add_dep_helper

F32 = mybir.dt.float32
BF16 = mybir.dt.bfloat16
FP8 = mybir.dt.float8e4
AF = mybir.ActivationFunctionType
OP = mybir.AluOpType
DR = mybir.MatmulPerfMode.DoubleRow
BF = ml_dtypes.bfloat16
F8 = mybir.dt.np(FP8)

B, LQ, LK, E, H, F = 4, 1024, 1024, 1024, 16, 4096
DH = E // H          # 64
R = 512              # query rows per core
SCALE = 1.0 / math.sqrt(DH)
HW = DH + 1          # head width in augmented v (64 dims + denom ones col)
N_CORES = 8
EPS = 1e-5

# fp8 static scales (powers of two; relative error is scale-invariant, these
# just keep everything inside e4m3's [2^-6, 240] sweet spot)
S_A = 16.0     # activations: t2T, t3T, memT, qT, kT
S_W = 512.0    # weights: wq wk wva wo w1 (~0.02 rms -> ~10)
S_E = 8.0      # softmax exp values
S_V = 32.0     # v
S_O = 64.0     # attention output rows


class PatchedTileContext(tile.TileContext):
    """Splits the kernel-tail drain's semaphore waits into individual wait_ge
    instructions; the installed walrus rejects >2 sync waits per instruction."""

    def _drain_and_barrier(self, tick_clock, wait_clock):
        nc = self.nc
        nop_inst = nc.sync.nop()
        wait_clock.add_sem_waits(
            nop_inst.ins, ScopedClock({None: tick_clock.global_clock})
        )
        mi = nop_inst.ins
        waits = list(mi.sync_info.on_wait) if (mi.sync_info and mi.sync_info.on_wait) else []
        if mi.sync_info is not None:
            mi.sync_info.on_wait = []
        assert self.sems is not None
        sem_by_id = {s.num: s for s in self.sems.allocated().values()}
        for w in waits:
            sem = sem_by_id.get(w.id)
            assert sem is not None, f"no sem handle for wait {w}"
            nc.sync.wait_ge(sem, w.wait_value)
        nc.sync.drain()

        nc.all_engine_barrier()
        popped = nc._tile_sem_poison_stack.pop()
        assert popped is self._sem_poison
        nc.clear_and_free_semaphores(list(self.sems.allocated().values()))
        nc.all_engine_barrier()


def _emit_layernorm_T(nc, xT, g_t, b_t, outT, ones_col_f32, ones_col_bf,
                      ones_row_f32, sq_pool, tmp_pool, stat_pool, bcast_pool,
                      small_pool, eps_t):
    """LayerNorm over features of a transposed activation.

    xT:   SBUF [128, 8, 512] f32   (feature-major; feature f = 128*t + p)
    outT: SBUF [128, 8, 512]       normalized * g + b (g/b host-prescaled)
    Row stats come from ones-vector matmuls (partition+tile reduction in one
    PSUM chain); A=rstd / B=-mean*rstd are broadcast to 128 partitions with a
    rank-1 ones matmul and applied as (x*A + B) * g + b.
    """
    ps_sum = stat_pool.tile([1, R], F32, tag="st_sum", name="ps_sum")
    ps_sq = stat_pool.tile([1, R], F32, tag="st_sq", name="ps_sq")
    for ti in range(8):
        sq_t = sq_pool.tile([128, R], BF16, tag="sq", name="sq_t")
        nc.scalar.activation(out=sq_t[:], in_=xT[:, ti, :], func=AF.Square)
        nc.tensor.matmul(ps_sum[:], ones_col_f32[:], xT[:, ti, :],
                         start=(ti == 0), stop=(ti == 7))
        nc.tensor.matmul(ps_sq[:], ones_col_bf[:], sq_t[:],
                         start=(ti == 0), stop=(ti == 7))
    mean = small_pool.tile([1, R], F32, tag="s0", name="mean")
    nc.vector.tensor_single_scalar(out=mean[:], in_=ps_sum[:], scalar=1.0 / E, op=OP.mult)
    ex2 = small_pool.tile([1, R], F32, tag="s1", name="ex2")
    nc.vector.tensor_single_scalar(out=ex2[:], in_=ps_sq[:], scalar=1.0 / E, op=OP.mult)
    msq = small_pool.tile([1, R], F32, tag="s2", name="msq")
    nc.vector.tensor_mul(out=msq[:], in0=mean[:], in1=mean[:])
    var = small_pool.tile([1, R], F32, tag="s3", name="var")
    nc.vector.tensor_sub(out=var[:], in0=ex2[:], in1=msq[:])
    lnv = small_pool.tile([1, R], F32, tag="s1", name="lnv")
    nc.scalar.activation(out=lnv[:], in_=var[:], func=AF.Ln, bias=eps_t[0:1, :])
    A = small_pool.tile([1, R], F32, tag="s2", name="A")
    nc.scalar.activation(out=A[:], in_=lnv[:], func=AF.Exp, scale=-0.5)
    Bv = small_pool.tile([1, R], F32, tag="s3", name="Bv")
    nc.vector.scalar_tensor_tensor(out=Bv[:], in0=mean[:], scalar=-1.0, in1=A[:],
                                   op0=OP.mult, op1=OP.mult)
    ps_A = bcast_pool.tile([128, R], F32, tag="bc_A", name="ps_A")
    ps_B = bcast_pool.tile([128, R], F32, tag="bc_B", name="ps_B")
    nc.tensor.matmul(ps_A[:], ones_row_f32[:], A[:], start=True, stop=True)
    nc.tensor.matmul(ps_B[:], ones_row_f32[:], Bv[:], start=True, stop=True)
    for ti in range(8):
        tmp = tmp_pool.tile([128, R], F32, tag="ln_tmp", name="tmp")
        nc.vector.tensor_mul(out=tmp[:], in0=xT[:, ti, :], in1=ps_A[:])
        tmp2 = tmp_pool.tile([128, R], F32, tag="ln_tmp2", name="tmp2")
        nc.vector.tensor_add(out=tmp2[:], in0=tmp[:], in1=ps_B[:])
        nc.scalar.activation(out=outT[:, ti, :], in_=tmp2[:], func=AF.Identity,
                             scale=g_t[:, ti:ti + 1], bias=b_t[:, ti:ti + 1])


def _split_sync_waits(nc, maxw=1):
    """Walrus rejects instructions carrying more than a couple of sync waits
    ("Too many sync wait commands"). Move excess waits onto NoOp instructions
    inserted just before, on the same engine queue — semantically identical
    (the engine blocks on the NoOp's wait before reaching the instruction)."""
    cnt = 0
    for f in nc.m.functions:
        for bb in f.blocks:
            insts = bb.instructions
            new_list = []
            for inst in insts:
                si = inst.sync_info
                waits = list(si.on_wait) if (si and si.on_wait) else []
                if len(waits) > maxw:
                    extra, keep = waits[:-maxw], waits[-maxw:]
                    si.on_wait = keep
                    for w in extra:
                        cnt += 1
                        nop = mybir.InstNoOp(
                            name=f"waitsplit-{cnt}", ins=[], outs=[],
                            engine=inst.engine,
                            sync_info=mybir.SyncInfo(on_wait=[w], on_update=[]))
                        new_list.append(nop)
                new_list.append(inst)
            insts[:] = new_list
    return cnt


def build_program():
    nc = bass.Bass("TRN2", target_bir_lowering=False, debug=False,
                   num_devices=N_CORES)

    # ---- DRAM parameters (per-core views, host-prepped) ----
    d_tgtT = nc.declare_dram_parameter("tgtT", [E, R], F32, isOutput=False)
    d_memT = nc.declare_dram_parameter("memT", [E, LK], FP8, isOutput=False)
    d_wq = nc.declare_dram_parameter("wq", [E, E], FP8, isOutput=False)
    d_wk = nc.declare_dram_parameter("wk", [E, E], FP8, isOutput=False)
    d_wva = nc.declare_dram_parameter("wva", [E, H * HW], FP8, isOutput=False)
    d_wo = nc.declare_dram_parameter("wo", [E, E], FP8, isOutput=False)
    d_w1 = nc.declare_dram_parameter("w1", [E, F], FP8, isOutput=False)
    d_w2 = nc.declare_dram_parameter("w2", [F, E], BF16, isOutput=False)
    d_bq = nc.declare_dram_parameter("bqp", [128, 8], F32, isOutput=False)
    d_bk = nc.declare_dram_parameter("bkp", [128, 8], F32, isOutput=False)
    d_bva = nc.declare_dram_parameter("bvap", [1, H * HW], BF16, isOutput=False)
    d_bo = nc.declare_dram_parameter("bor", [1, E], BF16, isOutput=False)
    d_b1 = nc.declare_dram_parameter("b1p", [128, 32], F32, isOutput=False)
    d_b2 = nc.declare_dram_parameter("b2p", [128, 8], F32, isOutput=False)
    d_g1 = nc.declare_dram_parameter("g1p", [128, 8], F32, isOutput=False)
    d_bb1 = nc.declare_dram_parameter("bb1p", [128, 8], F32, isOutput=False)
    d_g3 = nc.declare_dram_parameter("g3p", [128, 8], F32, isOutput=False)
    d_bb3 = nc.declare_dram_parameter("bb3p", [128, 8], F32, isOutput=False)
    d_outT = nc.declare_dram_parameter("outT", [E, R], F32, isOutput=True)
    d_attnT = nc.declare_dram_parameter("attnT", [LK, R], BF16, isOutput=True)
    d_cscr = nc.dram_tensor("cscratch", [H, R], BF16)

    with PatchedTileContext(nc) as tc, ExitStack() as top:
        consts = top.enter_context(tc.tile_pool(name="consts", bufs=1))
        persist = top.enter_context(tc.tile_pool(name="persist", bufs=1))

        # ---- constants / small parameter tiles ----
        ones_col_f32 = consts.tile([128, 1], F32)
        nc.vector.memset(ones_col_f32[:], 1.0)
        ones_col_bf = consts.tile([128, 1], BF16)
        nc.vector.memset(ones_col_bf[:], 1.0)
        ones_row_f32 = consts.tile([1, 128], F32)
        nc.vector.memset(ones_row_f32[:], 1.0)
        ones_sq_bf = consts.tile([128, 128], BF16)
        nc.vector.memset(ones_sq_bf[:], 1.0)
        ones_row512 = consts.tile([1, R], BF16)
        nc.vector.memset(ones_row512[:], 1.0)
        eps_t = consts.tile([128, 1], F32)
        nc.vector.memset(eps_t[:], EPS)
        ln8_t = consts.tile([128, 1], F32)
        nc.vector.memset(ln8_t[:], math.log(S_E))
        lnsh_t = consts.tile([128, 1], F32)
        nc.vector.memset(lnsh_t[:], math.log(S_V / H))

        bq_t = consts.tile([128, 8], F32)
        bk_t = consts.tile([128, 8], F32)
        bo_r = consts.tile([1, E], BF16)
        b1_t = consts.tile([128, 32], F32)
        b2_t = consts.tile([128, 8], F32)
        g1_t = consts.tile([128, 8], F32)
        bb1_t = consts.tile([128, 8], F32)
        g3_t = consts.tile([128, 8], F32)
        bb3_t = consts.tile([128, 8], F32)
        bva_t = consts.tile([1, H * HW], BF16)
        # Pool executes these tiny loads serially at ~0.6us each; they are
        # emitted inside phase 1 ordered by first use so the tgt tiles (which
        # gate LN1) lead the Pool ring.
        early_consts = ((g1_t, d_g1), (bb1_t, d_bb1), (bk_t, d_bk),
                        (bq_t, d_bq), (bva_t, d_bva))
        late_consts = ((bo_r, d_bo), (b1_t, d_b1), (b2_t, d_b2),
                       (g3_t, d_g3), (bb3_t, d_bb3))

        xT = persist.tile([128, 8, R], F32)

        # ===== right-stack prefetch pools =====
        # pfB (outer, lives through FFN): w1 first half + w2 first half
        # pfA (inner, dies after out-proj): attn output, wo, residual tgt
        pfB_stack = ExitStack()
        pfB = pfB_stack.enter_context(tc.tile_pool(name="pfB", bufs=1, side="right"))
        pfA_stack = ExitStack()
        pfA = pfA_stack.enter_context(tc.tile_pool(name="pfA", bufs=1, side="right"))

        attn_oT = pfA.tile([128, 8, R], FP8, tag="attn_oT", name="attn_oT")
        wo_t = pfA.tile([128, 8, E], FP8, tag="wo", name="wo_t")
        tgtr = pfA.tile([128, 8, R], F32, tag="tgtr", name="tgtr")
        w1h1 = pfB.tile([128, 8, F // 2], FP8, tag="w1h1", name="w1h1")

        # ===== attention-era tensors (live through phase 2) =====
        attn_stack = ExitStack()
        actp = attn_stack.enter_context(tc.tile_pool(name="actp", bufs=1))
        # projpA outlives the attention stack: after the last vchunk it holds
        # the first two out-proj chains, opened mid-attention as PE gap filler
        # and completed in phase 3.
        projp_stack = ExitStack()
        projpA = projp_stack.enter_context(tc.tile_pool(name="projpA", bufs=2, space="PSUM"))
        memT = actp.tile([128, 8, LK], FP8, tag="memT", name="memT")
        wk_t = actp.tile([128, 8, E], FP8, tag="wk", name="wk_t")
        wva_t = actp.tile([128, 8, H * HW], FP8, tag="wva", name="wva_t")
        qT = actp.tile([128, 8, R], FP8, tag="qT", name="qT")
        kT = actp.tile([128, 8, LK], FP8, tag="kT", name="kT")
        v_t = actp.tile([128, 8, H * HW], BF16, tag="vT", name="v_t")
        # dh-split copies of q/k: head h's 64 contraction dims rearranged as
        # [32 partitions, 2 DoubleRow k-tiles] so the scores matmul streams
        # two ifmap rows per cycle (fp8 DoubleRow). Pool executes these small
        # partition-shift copies off the hot engines. Rotating pools sized
        # for the ~4-head emission lookahead keep the SBUF cost at 18KB/part.
        qk2p = attn_stack.enter_context(tc.tile_pool(name="qk2p", bufs=6))
        q2t, k2t = {}, {}

        def emit_qshuf(h):
            ti, off = h // 2, (h % 2) * 64
            q2 = qk2p.tile([32, 2, R], FP8, tag="q2", name=f"q2_{h}")
            q2t[h] = q2
            for j in range(2):
                nc.gpsimd.dma_start(out=q2[:, j, :],
                                    in_=qT[off + 32 * j:off + 32 * (j + 1), ti, :])

        def emit_kshuf(h):
            ti, off = h // 2, (h % 2) * 64
            k2 = qk2p.tile([32, 2, LK], FP8, tag="k2", name=f"k2_{h}")
            k2t[h] = k2
            for j in range(2):
                nc.gpsimd.dma_start(out=k2[:, j, :],
                                    in_=kT[off + 32 * j:off + 32 * (j + 1), ti, :])

        CH = H * HW // 4  # 260 cols = 4 heads per v chunk

        def emit_kproj(to, shuf=True):
            for kc in range(2):
                ps = projpA.tile([128, R], F32, tag="proj", name="ps")
                for i in range(4):
                    nc.tensor.matmul(ps[:], wk_t[:, 2 * i:2 * i + 2, to * 128:(to + 1) * 128],
                                     memT[:, 2 * i:2 * i + 2, kc * 512:(kc + 1) * 512],
                                     perf_mode=DR, start=(i == 0), stop=(i == 3))
                nc.scalar.activation(out=kT[:, to, kc * 512:(kc + 1) * 512],
                                     in_=ps[:], func=AF.Identity,
                                     scale=1.0 / S_W, bias=bk_t[:, to:to + 1])
            if not shuf:
                return
            emit_kshuf(2 * to)
            emit_kshuf(2 * to + 1)
            if to >= 2:     # qT exists once phase 1's Q-proj has run
                emit_qshuf(2 * to)
                emit_qshuf(2 * to + 1)

        def emit_vchunk(ch):
            for kt in range(8):
                ps = projpA.tile([128, CH], F32, tag="proj", name="ps")
                for i in range(4):
                    nc.tensor.matmul(ps[:], memT[:, 2 * i:2 * i + 2, kt * 128:(kt + 1) * 128],
                                     wva_t[:, 2 * i:2 * i + 2, ch * CH:(ch + 1) * CH],
                                     perf_mode=DR, start=(i == 0), stop=False)
                nc.tensor.matmul(ps[:], ones_sq_bf[0:1, :],
                                 bva_t[:, ch * CH:(ch + 1) * CH],
                                 start=False, stop=True)
                # v = psum * S_V/(S_A*S_W)  (bias pre-amplified host-side).
                # ACT eviction: DVE is the attention bottleneck, and the Pool
                # engine cannot read PSUM.
                nc.scalar.activation(
                    out=v_t[:, kt, ch * CH:(ch + 1) * CH], in_=ps[:],
                    func=AF.Identity, scale=S_V / (S_A * S_W))

        # =========== Phase 1: LN1 + Q projection ===========
        qkv = ExitStack()
        with qkv:
            lnp = qkv.enter_context(tc.tile_pool(name="lnp", bufs=1))
            sqp = qkv.enter_context(tc.tile_pool(name="sqp", bufs=2))
            tmpp = qkv.enter_context(tc.tile_pool(name="tmpp", bufs=2))
            smallp = qkv.enter_context(tc.tile_pool(name="smallp", bufs=1))
            statp = qkv.enter_context(tc.tile_pool(name="statp", bufs=1, space="PSUM"))
            bcp = qkv.enter_context(tc.tile_pool(name="bcp", bufs=1, space="PSUM"))

            tgtT = lnp.tile([128, 8, R], F32, tag="tgtT", name="tgtT")
            wq_t = lnp.tile([128, 8, E], FP8, tag="wq", name="wq_t")
            # DMA emission order = per-ring hardware order. tgt (gates LN1)
            # is split 3-way and leads every ring; then each ring's loads
            # follow in first-use order.
            _tgt_src = d_tgtT[:].rearrange("(t p) r -> p t r", p=128)
            nc.sync.dma_start(out=tgtT[:, 0:3, :], in_=_tgt_src[:, 0:3, :])
            nc.sync.dma_start(out=tgtT[:, 3:6, :], in_=_tgt_src[:, 3:6, :])
            nc.gpsimd.dma_start(out=tgtT[:, 6:8, :], in_=_tgt_src[:, 6:8, :])
            for dst, src in early_consts:
                nc.gpsimd.dma_start(out=dst[:], in_=src[:])
            _mem_src = d_memT[:].rearrange("(t p) k -> p t k", p=128)
            _wk_src = d_wk[:].rearrange("(t p) o -> p t o", p=128)
            # chunked so kproj(0) can start before the full 2MB has landed
            nc.scalar.dma_start(out=memT[:, :, 0:512], in_=_mem_src[:, :, 0:512])
            nc.scalar.dma_start(out=wk_t[:, :, 0:512], in_=_wk_src[:, :, 0:512])
            nc.scalar.dma_start(out=memT[:, :, 512:1024], in_=_mem_src[:, :, 512:1024])
            nc.scalar.dma_start(out=wk_t[:, :, 512:1024], in_=_wk_src[:, :, 512:1024])
            nc.sync.dma_start(out=wq_t[:], in_=d_wq[:].rearrange("(t p) o -> p t o", p=128))
            nc.scalar.dma_start(out=wva_t[:], in_=d_wva[:].rearrange("(t p) o -> p t o", p=128))
            nc.scalar.dma_start(out=wo_t[:], in_=d_wo[:].rearrange("(t p) o -> p t o", p=128))
            nc.gpsimd.dma_start(out=tgtr[:], in_=_tgt_src[:])
            for dst, src in late_consts:
                nc.gpsimd.dma_start(out=dst[:], in_=src[:])
            nc.sync.dma_start(out=w1h1[:],
                              in_=d_w1[:, 0:F // 2].rearrange("(t p) f -> p t f", p=128))

            t2T = lnp.tile([128, 8, R], FP8, tag="t2T", name="t2T")
            _emit_layernorm_T(nc, tgtT, g1_t, bb1_t, t2T, ones_col_f32,
                              ones_col_bf, ones_row_f32, sqp, tmpp, statp,
                              bcp, smallp, eps_t)

            # K/V projections first: they only need memT/wk/wva (ready before
            # LN1's ACT/DVE chain finishes), so the PE queue isn't parked
            # behind Q-proj's wait on t2T.
            emit_kproj(0)
            emit_kproj(1)
            emit_vchunk(0)
            # kproj(2,3) fills the PE hole while LN1's apply chain produces
            # t2T; their q/k shuffles stay at the group-0 slot so the shuffle
            # pool's 6-buffer rotation depth is respected.
            emit_kproj(2, shuf=False)
            emit_kproj(3, shuf=False)

            for to in range(8):
                ps = projpA.tile([128, R], F32, tag="proj", name="ps")
                for i in range(4):
                    nc.tensor.matmul(ps[:], wq_t[:, 2 * i:2 * i + 2, to * 128:(to + 1) * 128],
                                     t2T[:, 2 * i:2 * i + 2, :], perf_mode=DR,
                                     start=(i == 0), stop=(i == 3))
                nc.scalar.activation(out=qT[:, to, :], in_=ps[:], func=AF.Identity,
                                     scale=1.0 / S_W, bias=bq_t[:, to:to + 1])

        for _h in range(4):
            emit_qshuf(_h)

        # =========== Phase 2: attention (16 heads, K/V interleaved) ===========
        att = ExitStack()
        with att:
            accp = att.enter_context(tc.tile_pool(name="accp", bufs=1))
            acc = [accp.tile([128, 2, R], BF16, tag=f"acc{kp}", name=f"acc{kp}")
                   for kp in range(4)]
            expp = att.enter_context(tc.tile_pool(name="expp", bufs=16))
            cbp = att.enter_context(tc.tile_pool(name="cbp", bufs=3))
            dnp = att.enter_context(tc.tile_pool(name="dnp", bufs=2))
            odtp = att.enter_context(tc.tile_pool(name="odtp", bufs=2))
            scps = att.enter_context(tc.tile_pool(name="scps", bufs=2, space="PSUM"))
            outps = att.enter_context(tc.tile_pool(name="outps", bufs=2, space="PSUM"))

            op_part = []
            deferred_acc = []
            for g in range(4):
                heads = list(range(4 * g, 4 * g + 4))
                if g == 3:
                    # end on an even head: its attn_oT eviction writes
                    # partitions 0:64 directly (no partition-shift DMA hop on
                    # the critical path into out-proj)
                    heads = [12, 13, 15, 14]
                for h in heads:
                    ti, off = h // 2, (h % 2) * 64
                    exp_ts = []
                    for ktp in range(4):
                        s_ps = scps.tile([128, 2, R], F32, tag="sc", name="s_ps")
                        for j in range(2):
                            kt = 2 * ktp + j
                            nc.tensor.matmul(
                                s_ps[:, j, :],
                                k2t[h][:, :, kt * 128:(kt + 1) * 128],
                                q2t[h][:],
                                perf_mode=DR, start=True, stop=True)
                        # e = S_E * exp(score/(S_A^2) * SCALE).  bf16, not fp8:
                        # the DVE head-mean muls read e_t, and fp8 operands run
                        # 1.65x slower on DVE — costing more than pv-DoubleRow
                        # saves on the PE (measured both ways).
                        e_t = expp.tile([128, 2, R], BF16, tag="exp", name="e_t")
                        nc.scalar.activation(out=e_t[:], in_=s_ps[:], func=AF.Exp,
                                             scale=SCALE / (S_A * S_A), bias=ln8_t[:])
                        exp_ts.append(e_t)

                    o_ps = outps.tile([128, R], F32, tag="o", name="o_ps")
                    for kt in range(8):
                        nc.tensor.matmul(o_ps[0:HW, :],
                                         v_t[:, kt, h * HW:(h + 1) * HW],
                                         exp_ts[kt // 2][:, kt % 2, :],
                                         start=(kt == 0), stop=(kt == 7))

                    # c = 1/denom via exp(-ln(d)) on ACT; plain DVE reciprocal
                    # costs 3.3us per call and walrus can't codegen the
                    # custom-op approx variants (InstISA).
                    dn_t = dnp.tile([128, R], F32, tag="dn", name="dn_t")
                    nc.scalar.activation(out=dn_t[64:65, :], in_=o_ps[64:65, :], func=AF.Ln)
                    # c = (S_V/H)/denom: the head-mean scale rides the exp bias
                    # so the accumulation mul is a plain tensor_mul (the 2-op
                    # scalar_tensor_tensor runs ~1.6x slower on DVE).
                    c_t = dnp.tile([128, R], F32, tag="c", name="c_t")
                    nc.scalar.activation(out=c_t[64:65, :], in_=dn_t[64:65, :], func=AF.Exp,
                                         scale=-1.0, bias=lnsh_t[64:65, :])
                    # broadcast c to all partitions: bounce through DRAM with
                    # a stride-0 partition source AP (idle DMA engines; frees
                    # PSUM banks vs a ones-matmul broadcast). Explicit dep
                    # edges order the loads behind the store.
                    cb = cbp.tile([128, 2, R], BF16, tag="cbt", name="cb")
                    st = nc.gpsimd.dma_start(out=d_cscr[h:h + 1, :], in_=c_t[64:65, :])
                    _sb = d_cscr[h:h + 1, :]
                    bc_ap = bass.AP(tensor=_sb.tensor, offset=_sb.offset,
                                    ap=[[0, 128], [1, R]])
                    ld1 = nc.gpsimd.dma_start(out=cb[:, 0, :], in_=bc_ap)
                    add_dep_helper(ld1.ins, st.ins, sync=True, reason="cb bcast after store")
                    ld2 = nc.gpsimd.dma_start(out=cb[:, 1, :], in_=bc_ap)
                    add_dep_helper(ld2.ins, st.ins, sync=True, reason="cb bcast after store")

                    # normalized per-head attention output rows:
                    # attn_oT = S_O * o = (o_ps * S_O) * cb
                    SO2 = S_O * H / S_V   # S_O / SH
                    if off == 0:
                        nc.vector.scalar_tensor_tensor(
                            out=attn_oT[0:64, ti, :], in0=o_ps[0:64, :], scalar=SO2,
                            in1=cb[0:64, 0, :], op0=OP.mult, op1=OP.mult)
                    else:
                        od_t = odtp.tile([64, R], FP8, tag="od", name="od_t")
                        nc.vector.scalar_tensor_tensor(
                            out=od_t[:], in0=o_ps[0:64, :], scalar=SO2,
                            in1=cb[0:64, 0, :], op0=OP.mult, op1=OP.mult)
                        nc.gpsimd.dma_start(out=attn_oT[64:128, ti, :], in_=od_t[:])

                    # head-mean accumulation: p/H = e_t * cb (scale in cb).
                    # The last three heads' accumulation gates nothing in
                    # out-proj, so it is emitted after the final attn_oT
                    # eviction — which then jumps a ~6us DVE backlog.
                    if h in (13, 15, 14):
                        deferred_acc.append((exp_ts, cb))
                    else:
                        for ktp in range(4):
                            if h == 0:
                                nc.vector.tensor_mul(
                                    out=acc[ktp][:], in0=exp_ts[ktp][:], in1=cb[:])
                            else:
                                tmp = cbp.tile([128, 2, R], BF16, tag="acctmp", name="tmp")
                                nc.vector.tensor_mul(
                                    out=tmp[:], in0=exp_ts[ktp][:], in1=cb[:])
                                nc.vector.tensor_add(out=acc[ktp][:], in0=acc[ktp][:], in1=tmp[:])

                    # Group 3 has no K/V filler left; park the first two
                    # out-proj chains' early contraction (heads 0-11, all
                    # evicted a head ago) in the PE queue as gap filler.
                    # h==12 (not later) so the queue never stalls on a
                    # just-written attn_oT tile.
                    if h == 12:
                        for to in range(2):
                            ps = projpA.tile([128, R], F32, tag="proj",
                                             name=f"op_part{to}")
                            op_part.append(ps)
                            for i in range(3):
                                nc.tensor.matmul(
                                    ps[:],
                                    wo_t[:, 2 * i:2 * i + 2, to * 128:(to + 1) * 128],
                                    attn_oT[:, 2 * i:2 * i + 2, :],
                                    perf_mode=DR, start=(i == 0), stop=False)

                # K/V projections for the next head group fill PE gaps while
                # ACT/DVE digest this group's softmax work
                if g == 0:
                    for _h2 in range(4, 8):
                        emit_kshuf(_h2)
                        emit_qshuf(_h2)
                elif g < 3:
                    emit_kproj(2 * g + 2)
                    emit_kproj(2 * g + 3)
                emit_vchunk(g + 1) if g < 3 else None

            # deferred head-mean accumulation (heads 13/15/14), then stores
            for exp_ts_d, cb_d in deferred_acc:
                for ktp in range(4):
                    tmp = cbp.tile([128, 2, R], BF16, tag="acctmp", name="tmp")
                    nc.vector.tensor_mul(out=tmp[:], in0=exp_ts_d[ktp][:], in1=cb_d[:])
                    nc.vector.tensor_add(out=acc[ktp][:], in0=acc[ktp][:], in1=tmp[:])

            # attn output store (bf16; upcast on host)
            for kp in range(4):
                nc.gpsimd.dma_start(out=d_attnT[(2 * kp) * 128:(2 * kp + 1) * 128, :],
                                    in_=acc[kp][:, 0, :])
                nc.gpsimd.dma_start(out=d_attnT[(2 * kp + 1) * 128:(2 * kp + 2) * 128, :],
                                    in_=acc[kp][:, 1, :])

        attn_stack.close()

        # w1's second half and all of w2 reuse the attention era's SBUF space.
        # w2h1 is allocated first: it lands on the earliest-dying attention
        # tiles (memT/wk/wva/qT die mid-attention), so its DMA starts during
        # head group 3 already.
        tails = ExitStack()
        tailp = tails.enter_context(tc.tile_pool(name="tailp", bufs=1))
        w2h1 = tailp.tile([128, 16, E], BF16, tag="w2h1", name="w2h1")
        nc.sync.dma_start(out=w2h1[:],
                          in_=d_w2[0:F // 2, :].rearrange("(t p) o -> p t o", p=128))
        w1h2 = tailp.tile([128, 8, F // 2], FP8, tag="w1h2", name="w1h2")
        nc.sync.dma_start(out=w1h2[:],
                          in_=d_w1[:, F // 2:F].rearrange("(t p) f -> p t f", p=128))
        w2h2 = tailp.tile([128, 16, E], BF16, tag="w2h2", name="w2h2")
        nc.scalar.dma_start(out=w2h2[:],
                            in_=d_w2[F // 2:F, :].rearrange("(t p) o -> p t o", p=128))
        hT = tailp.tile([128, 32, R], BF16, tag="hT", name="hT")
        t3T = tailp.tile([128, 8, R], FP8, tag="t3T", name="t3T")

        # ==== Phase 3: out-proj + residual, LN3 stats pipelined behind it ====
        p3 = ExitStack()
        with p3:
            opsp = p3.enter_context(tc.tile_pool(name="ops", bufs=2, space="PSUM"))
            sqp4 = p3.enter_context(tc.tile_pool(name="sqp4", bufs=2))
            tmpp4 = p3.enter_context(tc.tile_pool(name="tmpp4", bufs=2))
            smallp4 = p3.enter_context(tc.tile_pool(name="smallp4", bufs=1))
            statp4 = p3.enter_context(tc.tile_pool(name="statp4", bufs=1, space="PSUM"))
            bcp4 = p3.enter_context(tc.tile_pool(name="bcp4", bufs=1, space="PSUM"))
            ps_sum = statp4.tile([1, R], F32, tag="st_sum", name="ps_sum")
            ps_sq = statp4.tile([1, R], F32, tag="st_sq", name="ps_sq")

            def ln3_stats(ti):
                sq_t = sqp4.tile([128, R], BF16, tag="sq", name="sq_t")
                nc.scalar.activation(out=sq_t[:], in_=xT[:, ti, :], func=AF.Square)
                nc.tensor.matmul(ps_sum[:], ones_col_f32[:], xT[:, ti, :],
                                 start=(ti == 0), stop=(ti == 7))
                nc.tensor.matmul(ps_sq[:], ones_col_bf[:], sq_t[:],
                                 start=(ti == 0), stop=(ti == 7))

            for to in range(8):
                if to < 2:
                    ps = op_part[to]   # chains 0/1 opened during attention
                    ii = [3]
                else:
                    ps = opsp.tile([128, R], F32, tag="op", name="ps")
                    ii = [0, 1, 2, 3]
                for i in ii:
                    nc.tensor.matmul(ps[:], wo_t[:, 2 * i:2 * i + 2, to * 128:(to + 1) * 128],
                                     attn_oT[:, 2 * i:2 * i + 2, :], perf_mode=DR,
                                     start=(i == 0), stop=False)
                # rank-1 bias add: bo (host-amplified by S_O*S_W) via ones row
                nc.tensor.matmul(ps[:], bo_r[:, to * 128:(to + 1) * 128],
                                 ones_row512[:], start=False, stop=True)
                nc.vector.scalar_tensor_tensor(
                    out=xT[:, to, :], in0=ps[:], scalar=1.0 / (S_O * S_W),
                    in1=tgtr[:, to, :], op0=OP.mult, op1=OP.add)
                if to >= 1:
                    ln3_stats(to - 1)   # 1-tile lag so stats never park the PE
            ln3_stats(7)

            # LN3 tail: mean/var -> A/B -> broadcast -> apply
            mean = smallp4.tile([1, R], F32, tag="s0", name="mean")
            nc.vector.tensor_single_scalar(out=mean[:], in_=ps_sum[:], scalar=1.0 / E, op=OP.mult)
            ex2 = smallp4.tile([1, R], F32, tag="s1", name="ex2")
            nc.vector.tensor_single_scalar(out=ex2[:], in_=ps_sq[:], scalar=1.0 / E, op=OP.mult)
            msq = smallp4.tile([1, R], F32, tag="s2", name="msq")
            nc.vector.tensor_mul(out=msq[:], in0=mean[:], in1=mean[:])
            var = smallp4.tile([1, R], F32, tag="s3", name="var")
            nc.vector.tensor_sub(out=var[:], in0=ex2[:], in1=msq[:])
            lnv = smallp4.tile([1, R], F32, tag="s1", name="lnv")
            nc.scalar.activation(out=lnv[:], in_=var[:], func=AF.Ln, bias=eps_t[0:1, :])
            A = smallp4.tile([1, R], F32, tag="s2", name="A")
            nc.scalar.activation(out=A[:], in_=lnv[:], func=AF.Exp, scale=-0.5)
            Bv = smallp4.tile([1, R], F32, tag="s3", name="Bv")
            nc.vector.scalar_tensor_tensor(out=Bv[:], in0=mean[:], scalar=-1.0, in1=A[:],
                                           op0=OP.mult, op1=OP.mult)
            ps_A = bcp4.tile([128, R], F32, tag="bc_A", name="ps_A")
            ps_B = bcp4.tile([128, R], F32, tag="bc_B", name="ps_B")
            nc.tensor.matmul(ps_A[:], ones_row_f32[:], A[:], start=True, stop=True)
            nc.tensor.matmul(ps_B[:], ones_row_f32[:], Bv[:], start=True, stop=True)
            for ti in range(8):
                tmp = tmpp4.tile([128, R], F32, tag="ln_tmp", name="tmp")
                nc.vector.tensor_mul(out=tmp[:], in0=xT[:, ti, :], in1=ps_A[:])
                tmp2 = tmpp4.tile([128, R], F32, tag="ln_tmp2", name="tmp2")
                nc.vector.tensor_add(out=tmp2[:], in0=tmp[:], in1=ps_B[:])
                nc.scalar.activation(out=t3T[:, ti, :], in_=tmp2[:], func=AF.Identity,
                                     scale=g3_t[:, ti:ti + 1], bias=bb3_t[:, ti:ti + 1])
        projp_stack.close()
        pfA_stack.close()

        # =========== Phase 4: FFN ===========
        ffn = ExitStack()
        with ffn:
            fout = ffn.enter_context(tc.tile_pool(name="fout", bufs=3))
            ffnp = ffn.enter_context(tc.tile_pool(name="ffnp", bufs=4, space="PSUM"))

            for fo in range(32):
                w1h = w1h1 if fo < 16 else w1h2
                fl = fo % 16
                ps = ffnp.tile([128, R], F32, tag="ffn", name="ps")
                for i in range(4):
                    nc.tensor.matmul(ps[:], w1h[:, 2 * i:2 * i + 2, fl * 128:(fl + 1) * 128],
                                     t3T[:, 2 * i:2 * i + 2, :], perf_mode=DR,
                                     start=(i == 0), stop=(i == 3))
                nc.scalar.activation(out=hT[:, fo, :], in_=ps[:], func=AF.Relu,
                                     scale=1.0 / (S_A * S_W), bias=b1_t[:, fo:fo + 1])

            for eo in range(8):
                ps = ffnp.tile([128, R], F32, tag="ffn", name="ps")
                for fi in range(32):
                    w2h = w2h1 if fi < 16 else w2h2
                    nc.tensor.matmul(ps[:], w2h[:, fi % 16, eo * 128:(eo + 1) * 128],
                                     hT[:, fi, :], start=(fi == 0), stop=(fi == 31))
                fo_t = fout.tile([128, R], F32, tag="fo", name="fo_t")
                nc.vector.scalar_tensor_tensor(
                    out=fo_t[:], in0=ps[:], scalar=b2_t[:, eo:eo + 1],
                    in1=xT[:, eo, :], op0=OP.add, op1=OP.add)
                nc.sync.dma_start(out=d_outT[eo * 128:(eo + 1) * 128, :], in_=fo_t[:])
        tails.close()
        pfB_stack.close()
    _split_sync_waits(nc, maxw=1)
    return nc


_NC = None


def _get_program():
    global _NC
    if _NC is None:
        _NC = build_program()
    return _NC


def _prep_inputs(tgt, memory, ln1_g, ln1_b, wq, bq, wk, bk, wv, bv, wo, bo,
                 ln3_g, ln3_b, w1, b1, w2, b2):
    tgt = np.asarray(tgt, np.float32)
    memory = np.asarray(memory, np.float32)

    def part_tiles(vec, n, s=1.0):
        # [n*128] bias -> [128, n] per-partition tiles (feature f = 128*t + p)
        return np.ascontiguousarray(
            (np.asarray(vec, np.float32) * s).reshape(n, 128).T)

    def to8(x, s):
        return np.clip(np.asarray(x, np.float32) * s, -240.0, 240.0).astype(F8)

    wq_8 = to8(wq, S_W)
    wk_8 = to8(wk, S_W)
    wo_8 = to8(wo, S_W)
    w1_8 = to8(w1, S_W)
    w2_b = np.ascontiguousarray(np.asarray(w2, np.float32)).astype(BF)
    # augmented v-projection: per head 64 value cols + 1 zero col whose bias is 1
    wva = np.zeros((E, H * HW), np.float32)
    bva = np.zeros((1, H * HW), np.float32)
    wv_f = np.asarray(wv, np.float32)
    bv_f = np.asarray(bv, np.float32)
    for h in range(H):
        wva[:, h * HW:h * HW + DH] = wv_f[:, h * DH:(h + 1) * DH]
        bva[0, h * HW:h * HW + DH] = bv_f[h * DH:(h + 1) * DH]
        bva[0, h * HW + DH] = 1.0
    wva_8 = to8(wva, S_W)
    bva_b = (bva * (S_A * S_W)).astype(BF)     # added pre-dequant in PSUM
    bo_r = (np.asarray(bo, np.float32).reshape(1, E) * (S_O * S_W)).astype(BF)

    shared = {
        "wq": wq_8, "wk": wk_8, "wva": wva_8, "wo": wo_8,
        "w1": w1_8, "w2": w2_b,
        "bqp": part_tiles(bq, 8, S_A), "bkp": part_tiles(bk, 8, S_A),
        "bvap": bva_b, "bor": bo_r,
        "b1p": part_tiles(b1, 32), "b2p": part_tiles(b2, 8),
        "g1p": part_tiles(ln1_g, 8, S_A), "bb1p": part_tiles(ln1_b, 8, S_A),
        "g3p": part_tiles(ln3_g, 8, S_A), "bb3p": part_tiles(ln3_b, 8, S_A),
    }
    in_maps = []
    for c in range(N_CORES):
        b, hh = c // 2, c % 2
        rows = tgt[b, hh * R:(hh + 1) * R]            # [512, 1024]
        m = {"tgtT": np.ascontiguousarray(rows.T),
             "memT": np.ascontiguousarray(to8(memory[b].T, S_A))}
        m.update(shared)
        in_maps.append(m)
    return in_maps


def kernel(tgt, memory, ln1_g, ln1_b, wq, bq, wk, bk, wv, bv, wo, bo,
           ln3_g, ln3_b, w1, b1, w2, b2):
    in_maps = _prep_inputs(tgt, memory, ln1_g, ln1_b, wq, bq, wk, bk, wv, bv,
                           wo, bo, ln3_g, ln3_b, w1, b1, w2, b2)
    nc = _get_program()
    res = run_bass_kernel_spmd(nc, in_maps, list(range(N_CORES)))

    out = np.empty((B, LQ, E), np.float32)
    attn = np.empty((B, LQ, LK), np.float32)
    for c in range(N_CORES):
        b, hh = c // 2, c % 2
        out[b, hh * R:(hh + 1) * R] = res.results[c]["outT"].T
        attn[b, hh * R:(hh + 1) * R] = np.asarray(res.results[c]["attnT"],
                                                  np.float32).T
    return out, attn


# revision 84
# speedup vs baseline: 1.1541x; 1.1541x over previous
"""Trainium2 Bass kernel for a pre-norm cross-attention transformer layer.

Reference computation (B=4, Lq=Lk=1024, E=1024, H=16, Dh=64, F=4096):
    t2 = LN(tgt); q = t2@wq+bq; k = mem@wk+bk; v = mem@wv+bv
    p = softmax(q k^T / sqrt(Dh)); attn = mean_h(p)
    x = tgt + (p v)@wo + bo
    out = x + relu(LN(x)@w1+b1)@w2 + b2
Returns (out, attn).

Sharding: 8 cores = 4 batches x 2 query-halves. Each core owns 512 query rows
of one batch, computes K/V for its batch's full memory (duplicated within the
pair), and produces disjoint slices of both outputs -> no collectives.

On-device layout: all activations are kept transposed ([features, rows]) so
every matmul's contraction dim sits on SBUF partitions. The host passes
pre-transposed inputs and un-transposes outputs.

Precision: the q/k/v/o projections, the scores matmul, and FFN1 run in fp8
e4m3 with power-of-2 static scales folded into the PSUM-eviction activations;
fp8 pairs of contraction k-tiles are fed to the PE in DoubleRow perf mode
(two ifmap rows per cycle). The scores matmul gets its DoubleRow pair from a
Pool-engine partition-shift copy of q/k into [32, 2, .] layout (the 64-dim
head contraction split in half). FFN2 stays bf16: its operands' quantization
would add ~1.9% output error on its own, while the whole attention side adds
only ~0.1% (residual-dominated outputs). The softmax probabilities stay bf16
(p@v, head-mean): fp8 operands run 1.65x slower on the DVE, which is the
attention-phase bottleneck. Accumulation is fp32 in PSUM; softmax/LN
bookkeeping is fp32. Measured end-to-end: out rel err 1.88e-2, attn 8.3e-3
(gate 2e-2).

Softmax denominators come for free: wv is host-augmented with one extra
all-zero column per head whose bias row is 1, so the p@v accumulation's 65th
output row is sum_k exp(score). 1/denom is exp(-ln d) on ACT (the plain DVE
reciprocal costs 3.3us/call; walrus cannot codegen the custom-op approx
variants), with the head-mean scale folded into the exp bias so the
mean-accumulation is a plain tensor_mul.
"""

import math
import os
import sys
from contextlib import ExitStack

for _p in ("/opt/trn_rl_repo", "/root/.axon_site/_ro/trn_rl_repo"):
    if os.path.isdir(_p) and _p not in sys.path:
        sys.path.append(_p)

import ml_dtypes
import numpy as np

import concourse.bass as bass
import concourse.tile as tile
from concourse import mybir
from concourse.bass_utils import run_bass_kernel_spmd
from concourse.vector_clock import ScopedClock
from concourse.tile import add_dep_helper

F32 = mybir.dt.float32
BF16 = mybir.dt.bfloat16
FP8 = mybir.dt.float8e4
AF = mybir.ActivationFunctionType
OP = mybir.AluOpType
DR = mybir.MatmulPerfMode.DoubleRow
BF = ml_dtypes.bfloat16
F8 = mybir.dt.np(FP8)

B, LQ, LK, E, H, F = 4, 1024, 1024, 1024, 16, 4096
DH = E // H          # 64
R = 512              # query rows per core
SCALE = 1.0 / math.sqrt(DH)
HW = DH + 1          # head width in augmented v (64 dims + denom ones col)
N_CORES = 8
EPS = 1e-5

# fp8 static scales (powers of two; relative error is scale-invariant, these
# just keep everything inside e4m3's [2^-6, 240] sweet spot)
S_A = 16.0     # activations: t2T, t3T, memT, qT, kT
S_W = 512.0    # weights: wq wk wva wo w1 (~0.02 rms -> ~10)
S_E = 8.0      # softmax exp values
S_V = 32.0     # v
S_O = 64.0     # attention output rows


class PatchedTileContext(tile.TileContext):
    """Splits the kernel-tail drain's semaphore waits into individual wait_ge
    instructions; the installed walrus rejects >2 sync waits per instruction."""

    def _drain_and_barrier(self, tick_clock, wait_clock):
        nc = self.nc
        nop_inst = nc.sync.nop()
        wait_clock.add_sem_waits(
            nop_inst.ins, ScopedClock({None: tick_clock.global_clock})
        )
        mi = nop_inst.ins
        waits = list(mi.sync_info.on_wait) if (mi.sync_info and mi.sync_info.on_wait) else []
        if mi.sync_info is not None:
            mi.sync_info.on_wait = []
        assert self.sems is not None
        sem_by_id = {s.num: s for s in self.sems.allocated().values()}
        for w in waits:
            sem = sem_by_id.get(w.id)
            assert sem is not None, f"no sem handle for wait {w}"
            nc.sync.wait_ge(sem, w.wait_value)
        nc.sync.drain()

        nc.all_engine_barrier()
        popped = nc._tile_sem_poison_stack.pop()
        assert popped is self._sem_poison
        nc.clear_and_free_semaphores(list(self.sems.allocated().values()))
        nc.all_engine_barrier()


def _emit_layernorm_T(nc, xT, g_t, b_t, outT, ones_col_f32, ones_col_bf,
                      ones_row_f32, sq_pool, tmp_pool, stat_pool, bcast_pool,
                      small_pool, eps_t):
    """LayerNorm over features of a transposed activation.

    xT:   SBUF [128, 8, 512] f32   (feature-major; feature f = 128*t + p)
    outT: SBUF [128, 8, 512]       normalized * g + b (g/b host-prescaled)
    Row stats come from ones-vector matmuls (partition+tile reduction in one
    PSUM chain); A=rstd / B=-mean*rstd are broadcast to 128 partitions with a
    rank-1 ones matmul and applied as (x*A + B) * g + b.
    """
    ps_sum = stat_pool.tile([1, R], F32, tag="st_sum", name="ps_sum")
    ps_sq = stat_pool.tile([1, R], F32, tag="st_sq", name="ps_sq")
    for ti in range(8):
        sq_t = sq_pool.tile([128, R], BF16, tag="sq", name="sq_t")
        nc.scalar.activation(out=sq_t[:], in_=xT[:, ti, :], func=AF.Square)
        nc.tensor.matmul(ps_sum[:], ones_col_f32[:], xT[:, ti, :],
                         start=(ti == 0), stop=(ti == 7))
        nc.tensor.matmul(ps_sq[:], ones_col_bf[:], sq_t[:],
                         start=(ti == 0), stop=(ti == 7))
    mean = small_pool.tile([1, R], F32, tag="s0", name="mean")
    nc.vector.tensor_single_scalar(out=mean[:], in_=ps_sum[:], scalar=1.0 / E, op=OP.mult)
    ex2 = small_pool.tile([1, R], F32, tag="s1", name="ex2")
    nc.vector.tensor_single_scalar(out=ex2[:], in_=ps_sq[:], scalar=1.0 / E, op=OP.mult)
    msq = small_pool.tile([1, R], F32, tag="s2", name="msq")
    nc.vector.tensor_mul(out=msq[:], in0=mean[:], in1=mean[:])
    var = small_pool.tile([1, R], F32, tag="s3", name="var")
    nc.vector.tensor_sub(out=var[:], in0=ex2[:], in1=msq[:])
    lnv = small_pool.tile([1, R], F32, tag="s1", name="lnv")
    nc.scalar.activation(out=lnv[:], in_=var[:], func=AF.Ln, bias=eps_t[0:1, :])
    A = small_pool.tile([1, R], F32, tag="s2", name="A")
    nc.scalar.activation(out=A[:], in_=lnv[:], func=AF.Exp, scale=-0.5)
    Bv = small_pool.tile([1, R], F32, tag="s3", name="Bv")
    nc.vector.scalar_tensor_tensor(out=Bv[:], in0=mean[:], scalar=-1.0, in1=A[:],
                                   op0=OP.mult, op1=OP.mult)
    ps_A = bcast_pool.tile([128, R], F32, tag="bc_A", name="ps_A")
    ps_B = bcast_pool.tile([128, R], F32, tag="bc_B", name="ps_B")
    nc.tensor.matmul(ps_A[:], ones_row_f32[:], A[:], start=True, stop=True)
    nc.tensor.matmul(ps_B[:], ones_row_f32[:], Bv[:], start=True, stop=True)
    for ti in range(8):
        tmp = tmp_pool.tile([128, R], F32, tag="ln_tmp", name="tmp")
        nc.vector.tensor_mul(out=tmp[:], in0=xT[:, ti, :], in1=ps_A[:])
        tmp2 = tmp_pool.tile([128, R], F32, tag="ln_tmp2", name="tmp2")
        nc.vector.tensor_add(out=tmp2[:], in0=tmp[:], in1=ps_B[:])
        nc.scalar.activation(out=outT[:, ti, :], in_=tmp2[:], func=AF.Identity,
                             scale=g_t[:, ti:ti + 1], bias=b_t[:, ti:ti + 1])


def _split_sync_waits(nc, maxw=1):
    """Walrus rejects instructions carrying more than a couple of sync waits
    ("Too many sync wait commands"). Move excess waits onto NoOp instructions
    inserted just before, on the same engine queue — semantically identical
    (the engine blocks on the NoOp's wait before reaching the instruction)."""
    cnt = 0
    for f in nc.m.functions:
        for bb in f.blocks:
            insts = bb.instructions
            new_list = []
            for inst in insts:
                si = inst.sync_info
                waits = list(si.on_wait) if (si and si.on_wait) else []
                if len(waits) > maxw:
                    extra, keep = waits[:-maxw], waits[-maxw:]
                    si.on_wait = keep
                    for w in extra:
                        cnt += 1
                        nop = mybir.InstNoOp(
                            name=f"waitsplit-{cnt}", ins=[], outs=[],
                            engine=inst.engine,
                            sync_info=mybir.SyncInfo(on_wait=[w], on_update=[]))
                        new_list.append(nop)
                new_list.append(inst)
            insts[:] = new_list
    return cnt


def build_program():
    nc = bass.Bass("TRN2", target_bir_lowering=False, debug=False,
                   num_devices=N_CORES)

    # ---- DRAM parameters (per-core views, host-prepped) ----
    d_tgtT = nc.declare_dram_parameter("tgtT", [E, R], F32, isOutput=False)
    d_memT = nc.declare_dram_parameter("memT", [E, LK], FP8, isOutput=False)
    d_wq = nc.declare_dram_parameter("wq", [E, E], FP8, isOutput=False)
    d_wk = nc.declare_dram_parameter("wk", [E, E], FP8, isOutput=False)
    d_wva = nc.declare_dram_parameter("wva", [E, H * HW], FP8, isOutput=False)
    d_wo = nc.declare_dram_parameter("wo", [E, E], FP8, isOutput=False)
    d_w1 = nc.declare_dram_parameter("w1", [E, F], FP8, isOutput=False)
    d_w2 = nc.declare_dram_parameter("w2", [F, E], BF16, isOutput=False)
    d_bq = nc.declare_dram_parameter("bqp", [128, 8], F32, isOutput=False)
    d_bk = nc.declare_dram_parameter("bkp", [128, 8], F32, isOutput=False)
    d_bva = nc.declare_dram_parameter("bvap", [1, H * HW], BF16, isOutput=False)
    d_bo = nc.declare_dram_parameter("bor", [1, E], BF16, isOutput=False)
    d_b1 = nc.declare_dram_parameter("b1p", [128, 32], F32, isOutput=False)
    d_b2 = nc.declare_dram_parameter("b2p", [128, 8], F32, isOutput=False)
    d_g1 = nc.declare_dram_parameter("g1p", [128, 8], F32, isOutput=False)
    d_bb1 = nc.declare_dram_parameter("bb1p", [128, 8], F32, isOutput=False)
    d_g3 = nc.declare_dram_parameter("g3p", [128, 8], F32, isOutput=False)
    d_bb3 = nc.declare_dram_parameter("bb3p", [128, 8], F32, isOutput=False)
    d_outT = nc.declare_dram_parameter("outT", [E, R], F32, isOutput=True)
    d_attnT = nc.declare_dram_parameter("attnT", [LK, R], BF16, isOutput=True)
    d_cscr = nc.dram_tensor("cscratch", [H, R], BF16)

    with PatchedTileContext(nc) as tc, ExitStack() as top:
        consts = top.enter_context(tc.tile_pool(name="consts", bufs=1))
        persist = top.enter_context(tc.tile_pool(name="persist", bufs=1))

        # ---- constants / small parameter tiles ----
        ones_col_f32 = consts.tile([128, 1], F32)
        nc.vector.memset(ones_col_f32[:], 1.0)
        ones_col_bf = consts.tile([128, 1], BF16)
        nc.vector.memset(ones_col_bf[:], 1.0)
        ones_row_f32 = consts.tile([1, 128], F32)
        nc.vector.memset(ones_row_f32[:], 1.0)
        ones_sq_bf = consts.tile([128, 128], BF16)
        nc.vector.memset(ones_sq_bf[:], 1.0)
        ones_row512 = consts.tile([1, R], BF16)
        nc.vector.memset(ones_row512[:], 1.0)
        eps_t = consts.tile([128, 1], F32)
        nc.vector.memset(eps_t[:], EPS)
        ln8_t = consts.tile([128, 1], F32)
        nc.vector.memset(ln8_t[:], math.log(S_E))
        lnsh_t = consts.tile([128, 1], F32)
        nc.vector.memset(lnsh_t[:], math.log(S_V / H))

        bq_t = consts.tile([128, 8], F32)
        bk_t = consts.tile([128, 8], F32)
        bo_r = consts.tile([1, E], BF16)
        b1_t = consts.tile([128, 32], F32)
        b2_t = consts.tile([128, 8], F32)
        g1_t = consts.tile([128, 8], F32)
        bb1_t = consts.tile([128, 8], F32)
        g3_t = consts.tile([128, 8], F32)
        bb3_t = consts.tile([128, 8], F32)
        bva_t = consts.tile([1, H * HW], BF16)
        # Pool executes these tiny loads serially at ~0.6us each; they are
        # emitted inside phase 1 ordered by first use so the tgt tiles (which
        # gate LN1) lead the Pool ring.
        early_consts = ((g1_t, d_g1), (bb1_t, d_bb1), (bk_t, d_bk),
                        (bq_t, d_bq), (bva_t, d_bva))
        late_consts = ((bo_r, d_bo), (b1_t, d_b1), (b2_t, d_b2),
                       (g3_t, d_g3), (bb3_t, d_bb3))

        xT = persist.tile([128, 8, R], F32)

        # ===== right-stack prefetch pools =====
        # pfB (outer, lives through FFN): w1 first half + w2 first half
        # pfA (inner, dies after out-proj): attn output, wo, residual tgt
        pfB_stack = ExitStack()
        pfB = pfB_stack.enter_context(tc.tile_pool(name="pfB", bufs=1, side="right"))
        pfA_stack = ExitStack()
        pfA = pfA_stack.enter_context(tc.tile_pool(name="pfA", bufs=1, side="right"))

        attn_oT = pfA.tile([128, 8, R], FP8, tag="attn_oT", name="attn_oT")
        wo_t = pfA.tile([128, 8, E], FP8, tag="wo", name="wo_t")
        tgtr = pfA.tile([128, 8, R], F32, tag="tgtr", name="tgtr")
        w1h1 = pfB.tile([128, 8, F // 2], FP8, tag="w1h1", name="w1h1")

        # ===== attention-era tensors (live through phase 2) =====
        attn_stack = ExitStack()
        actp = attn_stack.enter_context(tc.tile_pool(name="actp", bufs=1))
        # projpA outlives the attention stack: after the last vchunk it holds
        # the first two out-proj chains, opened mid-attention as PE gap filler
        # and completed in phase 3.
        projp_stack = ExitStack()
        projpA = projp_stack.enter_context(tc.tile_pool(name="projpA", bufs=2, space="PSUM"))
        memT = actp.tile([128, 8, LK], FP8, tag="memT", name="memT")
        wk_t = actp.tile([128, 8, E], FP8, tag="wk", name="wk_t")
        wva_t = actp.tile([128, 8, H * HW], FP8, tag="wva", name="wva_t")
        qT = actp.tile([128, 8, R], FP8, tag="qT", name="qT")
        kT = actp.tile([128, 8, LK], FP8, tag="kT", name="kT")
        v_t = actp.tile([128, 8, H * HW], BF16, tag="vT", name="v_t")
        # dh-split copies of q/k: head h's 64 contraction dims rearranged as
        # [32 partitions, 2 DoubleRow k-tiles] so the scores matmul streams
        # two ifmap rows per cycle (fp8 DoubleRow). Pool executes these small
        # partition-shift copies off the hot engines. Rotating pools sized
        # for the ~4-head emission lookahead keep the SBUF cost at 18KB/part.
        qk2p = attn_stack.enter_context(tc.tile_pool(name="qk2p", bufs=6))
        q2t, k2t = {}, {}

        def emit_qshuf(h):
            ti, off = h // 2, (h % 2) * 64
            q2 = qk2p.tile([32, 2, R], FP8, tag="q2", name=f"q2_{h}")
            q2t[h] = q2
            for j in range(2):
                nc.gpsimd.dma_start(out=q2[:, j, :],
                                    in_=qT[off + 32 * j:off + 32 * (j + 1), ti, :])

        def emit_kshuf(h):
            ti, off = h // 2, (h % 2) * 64
            k2 = qk2p.tile([32, 2, LK], FP8, tag="k2", name=f"k2_{h}")
            k2t[h] = k2
            for j in range(2):
                nc.gpsimd.dma_start(out=k2[:, j, :],
                                    in_=kT[off + 32 * j:off + 32 * (j + 1), ti, :])

        CH = H * HW // 4  # 260 cols = 4 heads per v chunk

        def emit_kproj(to, shuf=True):
            for kc in range(2):
                ps = projpA.tile([128, R], F32, tag="proj", name="ps")
                for i in range(4):
                    nc.tensor.matmul(ps[:], wk_t[:, 2 * i:2 * i + 2, to * 128:(to + 1) * 128],
                                     memT[:, 2 * i:2 * i + 2, kc * 512:(kc + 1) * 512],
                                     perf_mode=DR, start=(i == 0), stop=(i == 3))
                nc.scalar.activation(out=kT[:, to, kc * 512:(kc + 1) * 512],
                                     in_=ps[:], func=AF.Identity,
                                     scale=1.0 / S_W, bias=bk_t[:, to:to + 1])
            if not shuf:
                return
            emit_kshuf(2 * to)
            emit_kshuf(2 * to + 1)
            if to >= 2:     # qT exists once phase 1's Q-proj has run
                emit_qshuf(2 * to)
                emit_qshuf(2 * to + 1)

        def emit_vchunk(ch):
            for kt in range(8):
                ps = projpA.tile([128, CH], F32, tag="proj", name="ps")
                for i in range(4):
                    nc.tensor.matmul(ps[:], memT[:, 2 * i:2 * i + 2, kt * 128:(kt + 1) * 128],
                                     wva_t[:, 2 * i:2 * i + 2, ch * CH:(ch + 1) * CH],
                                     perf_mode=DR, start=(i == 0), stop=False)
                nc.tensor.matmul(ps[:], ones_sq_bf[0:1, :],
                                 bva_t[:, ch * CH:(ch + 1) * CH],
                                 start=False, stop=True)
                # v = psum * S_V/(S_A*S_W)  (bias pre-amplified host-side).
                # ACT eviction: DVE is the attention bottleneck, and the Pool
                # engine cannot read PSUM.
                nc.scalar.activation(
                    out=v_t[:, kt, ch * CH:(ch + 1) * CH], in_=ps[:],
                    func=AF.Identity, scale=S_V / (S_A * S_W))

        # =========== Phase 1: LN1 + Q projection ===========
        qkv = ExitStack()
        with qkv:
            lnp = qkv.enter_context(tc.tile_pool(name="lnp", bufs=1))
            sqp = qkv.enter_context(tc.tile_pool(name="sqp", bufs=2))
            tmpp = qkv.enter_context(tc.tile_pool(name="tmpp", bufs=2))
            smallp = qkv.enter_context(tc.tile_pool(name="smallp", bufs=1))
            statp = qkv.enter_context(tc.tile_pool(name="statp", bufs=1, space="PSUM"))
            bcp = qkv.enter_context(tc.tile_pool(name="bcp", bufs=1, space="PSUM"))

            tgtT = lnp.tile([128, 8, R], F32, tag="tgtT", name="tgtT")
            wq_t = lnp.tile([128, 8, E], FP8, tag="wq", name="wq_t")
            # DMA emission order = per-ring hardware order. tgt (gates LN1)
            # is split 3-way and leads every ring; then each ring's loads
            # follow in first-use order.
            _tgt_src = d_tgtT[:].rearrange("(t p) r -> p t r", p=128)
            nc.sync.dma_start(out=tgtT[:, 0:3, :], in_=_tgt_src[:, 0:3, :])
            nc.sync.dma_start(out=tgtT[:, 3:6, :], in_=_tgt_src[:, 3:6, :])
            nc.gpsimd.dma_start(out=tgtT[:, 6:8, :], in_=_tgt_src[:, 6:8, :])
            for dst, src in early_consts:
                nc.gpsimd.dma_start(out=dst[:], in_=src[:])
            _mem_src = d_memT[:].rearrange("(t p) k -> p t k", p=128)
            _wk_src = d_wk[:].rearrange("(t p) o -> p t o", p=128)
            # chunked so kproj(0) can start before the full 2MB has landed
            nc.scalar.dma_start(out=memT[:, :, 0:512], in_=_mem_src[:, :, 0:512])
            nc.scalar.dma_start(out=wk_t[:, :, 0:256], in_=_wk_src[:, :, 0:256])
            nc.scalar.dma_start(out=memT[:, :, 512:1024], in_=_mem_src[:, :, 512:1024])
            nc.scalar.dma_start(out=wk_t[:, :, 256:1024], in_=_wk_src[:, :, 256:1024])
            nc.sync.dma_start(out=wq_t[:], in_=d_wq[:].rearrange("(t p) o -> p t o", p=128))
            nc.scalar.dma_start(out=wva_t[:], in_=d_wva[:].rearrange("(t p) o -> p t o", p=128))
            nc.scalar.dma_start(out=wo_t[:], in_=d_wo[:].rearrange("(t p) o -> p t o", p=128))
            nc.gpsimd.dma_start(out=tgtr[:], in_=_tgt_src[:])
            for dst, src in late_consts:
                nc.gpsimd.dma_start(out=dst[:], in_=src[:])
            nc.sync.dma_start(out=w1h1[:],
                              in_=d_w1[:, 0:F // 2].rearrange("(t p) f -> p t f", p=128))

            t2T = lnp.tile([128, 8, R], FP8, tag="t2T", name="t2T")
            _emit_layernorm_T(nc, tgtT, g1_t, bb1_t, t2T, ones_col_f32,
                              ones_col_bf, ones_row_f32, sqp, tmpp, statp,
                              bcp, smallp, eps_t)

            # K/V projections first: they only need memT/wk/wva (ready before
            # LN1's ACT/DVE chain finishes), so the PE queue isn't parked
            # behind Q-proj's wait on t2T.
            emit_kproj(0)
            emit_kproj(1)
            emit_vchunk(0)
            # kproj(2,3) fills the PE hole while LN1's apply chain produces
            # t2T; their q/k shuffles stay at the group-0 slot so the shuffle
            # pool's 6-buffer rotation depth is respected.
            emit_kproj(2, shuf=False)
            emit_kproj(3, shuf=False)

            for to in range(8):
                ps = projpA.tile([128, R], F32, tag="proj", name="ps")
                for i in range(4):
                    nc.tensor.matmul(ps[:], wq_t[:, 2 * i:2 * i + 2, to * 128:(to + 1) * 128],
                                     t2T[:, 2 * i:2 * i + 2, :], perf_mode=DR,
                                     start=(i == 0), stop=(i == 3))
                nc.scalar.activation(out=qT[:, to, :], in_=ps[:], func=AF.Identity,
                                     scale=1.0 / S_W, bias=bq_t[:, to:to + 1])

        for _h in range(4):
            emit_qshuf(_h)

        # =========== Phase 2: attention (16 heads, K/V interleaved) ===========
        att = ExitStack()
        with att:
            accp = att.enter_context(tc.tile_pool(name="accp", bufs=1))
            acc = [accp.tile([128, 2, R], BF16, tag=f"acc{kp}", name=f"acc{kp}")
                   for kp in range(4)]
            expp = att.enter_context(tc.tile_pool(name="expp", bufs=16))
            cbp = att.enter_context(tc.tile_pool(name="cbp", bufs=3))
            dnp = att.enter_context(tc.tile_pool(name="dnp", bufs=2))
            odtp = att.enter_context(tc.tile_pool(name="odtp", bufs=2))
            scps = att.enter_context(tc.tile_pool(name="scps", bufs=2, space="PSUM"))
            outps = att.enter_context(tc.tile_pool(name="outps", bufs=2, space="PSUM"))

            op_part = []
            deferred_acc = []
            for g in range(4):
                heads = list(range(4 * g, 4 * g + 4))
                if g == 3:
                    # end on an even head: its attn_oT eviction writes
                    # partitions 0:64 directly (no partition-shift DMA hop on
                    # the critical path into out-proj)
                    heads = [12, 13, 15, 14]
                for h in heads:
                    ti, off = h // 2, (h % 2) * 64
                    exp_ts = []
                    for ktp in range(4):
                        s_ps = scps.tile([128, 2, R], F32, tag="sc", name="s_ps")
                        for j in range(2):
                            kt = 2 * ktp + j
                            nc.tensor.matmul(
                                s_ps[:, j, :],
                                k2t[h][:, :, kt * 128:(kt + 1) * 128],
                                q2t[h][:],
                                perf_mode=DR, start=True, stop=True)
                        # e = S_E * exp(score/(S_A^2) * SCALE).  bf16, not fp8:
                        # the DVE head-mean muls read e_t, and fp8 operands run
                        # 1.65x slower on DVE — costing more than pv-DoubleRow
                        # saves on the PE (measured both ways).
                        e_t = expp.tile([128, 2, R], BF16, tag="exp", name="e_t")
                        nc.scalar.activation(out=e_t[:], in_=s_ps[:], func=AF.Exp,
                                             scale=SCALE / (S_A * S_A), bias=ln8_t[:])
                        exp_ts.append(e_t)

                    o_ps = outps.tile([128, R], F32, tag="o", name="o_ps")
                    for kt in range(8):
                        nc.tensor.matmul(o_ps[0:HW, :],
                                         v_t[:, kt, h * HW:(h + 1) * HW],
                                         exp_ts[kt // 2][:, kt % 2, :],
                                         start=(kt == 0), stop=(kt == 7))

                    # c = 1/denom via exp(-ln(d)) on ACT; plain DVE reciprocal
                    # costs 3.3us per call and walrus can't codegen the
                    # custom-op approx variants (InstISA).
                    dn_t = dnp.tile([128, R], F32, tag="dn", name="dn_t")
                    nc.scalar.activation(out=dn_t[64:65, :], in_=o_ps[64:65, :], func=AF.Ln)
                    # c = (S_V/H)/denom: the head-mean scale rides the exp bias
                    # so the accumulation mul is a plain tensor_mul (the 2-op
                    # scalar_tensor_tensor runs ~1.6x slower on DVE).
                    c_t = dnp.tile([128, R], F32, tag="c", name="c_t")
                    nc.scalar.activation(out=c_t[64:65, :], in_=dn_t[64:65, :], func=AF.Exp,
                                         scale=-1.0, bias=lnsh_t[64:65, :])
                    # broadcast c to all partitions: bounce through DRAM with
                    # a stride-0 partition source AP (idle DMA engines; frees
                    # PSUM banks vs a ones-matmul broadcast). Explicit dep
                    # edges order the loads behind the store.
                    cb = cbp.tile([128, 2, R], BF16, tag="cbt", name="cb")
                    st = nc.gpsimd.dma_start(out=d_cscr[h:h + 1, :], in_=c_t[64:65, :])
                    _sb = d_cscr[h:h + 1, :]
                    bc_ap = bass.AP(tensor=_sb.tensor, offset=_sb.offset,
                                    ap=[[0, 128], [1, R]])
                    ld1 = nc.gpsimd.dma_start(out=cb[:, 0, :], in_=bc_ap)
                    add_dep_helper(ld1.ins, st.ins, sync=True, reason="cb bcast after store")
                    ld2 = nc.gpsimd.dma_start(out=cb[:, 1, :], in_=bc_ap)
                    add_dep_helper(ld2.ins, st.ins, sync=True, reason="cb bcast after store")

                    # normalized per-head attention output rows:
                    # attn_oT = S_O * o = (o_ps * S_O) * cb
                    SO2 = S_O * H / S_V   # S_O / SH
                    if off == 0:
                        nc.vector.scalar_tensor_tensor(
                            out=attn_oT[0:64, ti, :], in0=o_ps[0:64, :], scalar=SO2,
                            in1=cb[0:64, 0, :], op0=OP.mult, op1=OP.mult)
                    else:
                        od_t = odtp.tile([64, R], FP8, tag="od", name="od_t")
                        nc.vector.scalar_tensor_tensor(
                            out=od_t[:], in0=o_ps[0:64, :], scalar=SO2,
                            in1=cb[0:64, 0, :], op0=OP.mult, op1=OP.mult)
                        nc.gpsimd.dma_start(out=attn_oT[64:128, ti, :], in_=od_t[:])

                    # head-mean accumulation: p/H = e_t * cb (scale in cb).
                    # The last three heads' accumulation gates nothing in
                    # out-proj, so it is emitted after the final attn_oT
                    # eviction — which then jumps a ~6us DVE backlog.
                    if h in (13, 15, 14):
                        deferred_acc.append((exp_ts, cb))
                    else:
                        for ktp in range(4):
                            if h == 0:
                                nc.vector.tensor_mul(
                                    out=acc[ktp][:], in0=exp_ts[ktp][:], in1=cb[:])
                            else:
                                tmp = cbp.tile([128, 2, R], BF16, tag="acctmp", name="tmp")
                                nc.vector.tensor_mul(
                                    out=tmp[:], in0=exp_ts[ktp][:], in1=cb[:])
                                nc.vector.tensor_add(out=acc[ktp][:], in0=acc[ktp][:], in1=tmp[:])

                    # Group 3 has no K/V filler left; park the first two
                    # out-proj chains' early contraction (heads 0-11, all
                    # evicted a head ago) in the PE queue as gap filler.
                    # h==12 (not later) so the queue never stalls on a
                    # just-written attn_oT tile.
                    if h == 12:
                        for to in range(2):
                            ps = projpA.tile([128, R], F32, tag="proj",
                                             name=f"op_part{to}")
                            op_part.append(ps)
                            for i in range(3):
                                nc.tensor.matmul(
                                    ps[:],
                                    wo_t[:, 2 * i:2 * i + 2, to * 128:(to + 1) * 128],
                                    attn_oT[:, 2 * i:2 * i + 2, :],
                                    perf_mode=DR, start=(i == 0), stop=False)

                # K/V projections for the next head group fill PE gaps while
                # ACT/DVE digest this group's softmax work
                if g == 0:
                    for _h2 in range(4, 8):
                        emit_kshuf(_h2)
                        emit_qshuf(_h2)
                elif g < 3:
                    emit_kproj(2 * g + 2)
                    emit_kproj(2 * g + 3)
                emit_vchunk(g + 1) if g < 3 else None

            # deferred head-mean accumulation (heads 13/15/14), then stores
            for exp_ts_d, cb_d in deferred_acc:
                for ktp in range(4):
                    tmp = cbp.tile([128, 2, R], BF16, tag="acctmp", name="tmp")
                    nc.vector.tensor_mul(out=tmp[:], in0=exp_ts_d[ktp][:], in1=cb_d[:])
                    nc.vector.tensor_add(out=acc[ktp][:], in0=acc[ktp][:], in1=tmp[:])

            # attn output store (bf16; upcast on host)
            for kp in range(4):
                nc.gpsimd.dma_start(out=d_attnT[(2 * kp) * 128:(2 * kp + 1) * 128, :],
                                    in_=acc[kp][:, 0, :])
                nc.gpsimd.dma_start(out=d_attnT[(2 * kp + 1) * 128:(2 * kp + 2) * 128, :],
                                    in_=acc[kp][:, 1, :])

        attn_stack.close()

        # w1's second half and all of w2 reuse the attention era's SBUF space.
        # w2h1 is allocated first: it lands on the earliest-dying attention
        # tiles (memT/wk/wva/qT die mid-attention), so its DMA starts during
        # head group 3 already.
        tails = ExitStack()
        tailp = tails.enter_context(tc.tile_pool(name="tailp", bufs=1))
        w2h1 = tailp.tile([128, 16, E], BF16, tag="w2h1", name="w2h1")
        nc.sync.dma_start(out=w2h1[:],
                          in_=d_w2[0:F // 2, :].rearrange("(t p) o -> p t o", p=128))
        w1h2 = tailp.tile([128, 8, F // 2], FP8, tag="w1h2", name="w1h2")
        nc.sync.dma_start(out=w1h2[:],
                          in_=d_w1[:, F // 2:F].rearrange("(t p) f -> p t f", p=128))
        w2h2 = tailp.tile([128, 16, E], BF16, tag="w2h2", name="w2h2")
        nc.scalar.dma_start(out=w2h2[:],
                            in_=d_w2[F // 2:F, :].rearrange("(t p) o -> p t o", p=128))
        hT = tailp.tile([128, 32, R], BF16, tag="hT", name="hT")
        t3T = tailp.tile([128, 8, R], FP8, tag="t3T", name="t3T")

        # ==== Phase 3: out-proj + residual, LN3 stats pipelined behind it ====
        p3 = ExitStack()
        with p3:
            opsp = p3.enter_context(tc.tile_pool(name="ops", bufs=2, space="PSUM"))
            sqp4 = p3.enter_context(tc.tile_pool(name="sqp4", bufs=2))
            tmpp4 = p3.enter_context(tc.tile_pool(name="tmpp4", bufs=2))
            smallp4 = p3.enter_context(tc.tile_pool(name="smallp4", bufs=1))
            statp4 = p3.enter_context(tc.tile_pool(name="statp4", bufs=1, space="PSUM"))
            bcp4 = p3.enter_context(tc.tile_pool(name="bcp4", bufs=1, space="PSUM"))
            ps_sum = statp4.tile([1, R], F32, tag="st_sum", name="ps_sum")
            ps_sq = statp4.tile([1, R], F32, tag="st_sq", name="ps_sq")

            def ln3_stats(ti):
                sq_t = sqp4.tile([128, R], BF16, tag="sq", name="sq_t")
                nc.scalar.activation(out=sq_t[:], in_=xT[:, ti, :], func=AF.Square)
                nc.tensor.matmul(ps_sum[:], ones_col_f32[:], xT[:, ti, :],
                                 start=(ti == 0), stop=(ti == 7))
                nc.tensor.matmul(ps_sq[:], ones_col_bf[:], sq_t[:],
                                 start=(ti == 0), stop=(ti == 7))

            for to in range(8):
                if to < 2:
                    ps = op_part[to]   # chains 0/1 opened during attention
                    ii = [3]
                else:
                    ps = opsp.tile([128, R], F32, tag="op", name="ps")
                    ii = [0, 1, 2, 3]
                for i in ii:
                    nc.tensor.matmul(ps[:], wo_t[:, 2 * i:2 * i + 2, to * 128:(to + 1) * 128],
                                     attn_oT[:, 2 * i:2 * i + 2, :], perf_mode=DR,
                                     start=(i == 0), stop=False)
                # rank-1 bias add: bo (host-amplified by S_O*S_W) via ones row
                nc.tensor.matmul(ps[:], bo_r[:, to * 128:(to + 1) * 128],
                                 ones_row512[:], start=False, stop=True)
                nc.vector.scalar_tensor_tensor(
                    out=xT[:, to, :], in0=ps[:], scalar=1.0 / (S_O * S_W),
                    in1=tgtr[:, to, :], op0=OP.mult, op1=OP.add)
                if to >= 1:
                    ln3_stats(to - 1)   # 1-tile lag so stats never park the PE
            ln3_stats(7)

            # LN3 tail: mean/var -> A/B -> broadcast -> apply
            mean = smallp4.tile([1, R], F32, tag="s0", name="mean")
            nc.vector.tensor_single_scalar(out=mean[:], in_=ps_sum[:], scalar=1.0 / E, op=OP.mult)
            ex2 = smallp4.tile([1, R], F32, tag="s1", name="ex2")
            nc.vector.tensor_single_scalar(out=ex2[:], in_=ps_sq[:], scalar=1.0 / E, op=OP.mult)
            msq = smallp4.tile([1, R], F32, tag="s2", name="msq")
            nc.vector.tensor_mul(out=msq[:], in0=mean[:], in1=mean[:])
            var = smallp4.tile([1, R], F32, tag="s3", name="var")
            nc.vector.tensor_sub(out=var[:], in0=ex2[:], in1=msq[:])
            lnv = smallp4.tile([1, R], F32, tag="s1", name="lnv")
            nc.scalar.activation(out=lnv[:], in_=var[:], func=AF.Ln, bias=eps_t[0:1, :])
            A = smallp4.tile([1, R], F32, tag="s2", name="A")
            nc.scalar.activation(out=A[:], in_=lnv[:], func=AF.Exp, scale=-0.5)
            Bv = smallp4.tile([1, R], F32, tag="s3", name="Bv")
            nc.vector.scalar_tensor_tensor(out=Bv[:], in0=mean[:], scalar=-1.0, in1=A[:],
                                           op0=OP.mult, op1=OP.mult)
            ps_A = bcp4.tile([128, R], F32, tag="bc_A", name="ps_A")
            ps_B = bcp4.tile([128, R], F32, tag="bc_B", name="ps_B")
            nc.tensor.matmul(ps_A[:], ones_row_f32[:], A[:], start=True, stop=True)
            nc.tensor.matmul(ps_B[:], ones_row_f32[:], Bv[:], start=True, stop=True)
            for ti in range(8):
                tmp = tmpp4.tile([128, R], F32, tag="ln_tmp", name="tmp")
                nc.vector.tensor_mul(out=tmp[:], in0=xT[:, ti, :], in1=ps_A[:])
                tmp2 = tmpp4.tile([128, R], F32, tag="ln_tmp2", name="tmp2")
                nc.vector.tensor_add(out=tmp2[:], in0=tmp[:], in1=ps_B[:])
                nc.scalar.activation(out=t3T[:, ti, :], in_=tmp2[:], func=AF.Identity,
                                     scale=g3_t[:, ti:ti + 1], bias=bb3_t[:, ti:ti + 1])
        projp_stack.close()
        pfA_stack.close()

        # =========== Phase 4: FFN ===========
        ffn = ExitStack()
        with ffn:
            fout = ffn.enter_context(tc.tile_pool(name="fout", bufs=3))
            ffnp = ffn.enter_context(tc.tile_pool(name="ffnp", bufs=4, space="PSUM"))

            for fo in range(32):
                w1h = w1h1 if fo < 16 else w1h2
                fl = fo % 16
                ps = ffnp.tile([128, R], F32, tag="ffn", name="ps")
                for i in range(4):
                    nc.tensor.matmul(ps[:], w1h[:, 2 * i:2 * i + 2, fl * 128:(fl + 1) * 128],
                                     t3T[:, 2 * i:2 * i + 2, :], perf_mode=DR,
                                     start=(i == 0), stop=(i == 3))
                nc.scalar.activation(out=hT[:, fo, :], in_=ps[:], func=AF.Relu,
                                     scale=1.0 / (S_A * S_W), bias=b1_t[:, fo:fo + 1])

            for eo in range(8):
                ps = ffnp.tile([128, R], F32, tag="ffn", name="ps")
                for fi in range(32):
                    w2h = w2h1 if fi < 16 else w2h2
                    nc.tensor.matmul(ps[:], w2h[:, fi % 16, eo * 128:(eo + 1) * 128],
                                     hT[:, fi, :], start=(fi == 0), stop=(fi == 31))
                fo_t = fout.tile([128, R], F32, tag="fo", name="fo_t")
                nc.vector.scalar_tensor_tensor(
                    out=fo_t[:], in0=ps[:], scalar=b2_t[:, eo:eo + 1],
                    in1=xT[:, eo, :], op0=OP.add, op1=OP.add)
                nc.sync.dma_start(out=d_outT[eo * 128:(eo + 1) * 128, :], in_=fo_t[:])
        tails.close()
        pfB_stack.close()
    _split_sync_waits(nc, maxw=1)
    return nc


_NC = None


def _get_program():
    global _NC
    if _NC is None:
        _NC = build_program()
    return _NC


def _prep_inputs(tgt, memory, ln1_g, ln1_b, wq, bq, wk, bk, wv, bv, wo, bo,
                 ln3_g, ln3_b, w1, b1, w2, b2):
    tgt = np.asarray(tgt, np.float32)
    memory = np.asarray(memory, np.float32)

    def part_tiles(vec, n, s=1.0):
        # [n*128] bias -> [128, n] per-partition tiles (feature f = 128*t + p)
        return np.ascontiguousarray(
            (np.asarray(vec, np.float32) * s).reshape(n, 128).T)

    def to8(x, s):
        return np.clip(np.asarray(x, np.float32) * s, -240.0, 240.0).astype(F8)

    wq_8 = to8(wq, S_W)
    wk_8 = to8(wk, S_W)
    wo_8 = to8(wo, S_W)
    w1_8 = to8(w1, S_W)
    w2_b = np.ascontiguousarray(np.asarray(w2, np.float32)).astype(BF)
    # augmented v-projection: per head 64 value cols + 1 zero col whose bias is 1
    wva = np.zeros((E, H * HW), np.float32)
    bva = np.zeros((1, H * HW), np.float32)
    wv_f = np.asarray(wv, np.float32)
    bv_f = np.asarray(bv, np.float32)
    for h in range(H):
        wva[:, h * HW:h * HW + DH] = wv_f[:, h * DH:(h + 1) * DH]
        bva[0, h * HW:h * HW + DH] = bv_f[h * DH:(h + 1) * DH]
        bva[0, h * HW + DH] = 1.0
    wva_8 = to8(wva, S_W)
    bva_b = (bva * (S_A * S_W)).astype(BF)     # added pre-dequant in PSUM
    bo_r = (np.asarray(bo, np.float32).reshape(1, E) * (S_O * S_W)).astype(BF)

    shared = {
        "wq": wq_8, "wk": wk_8, "wva": wva_8, "wo": wo_8,
        "w1": w1_8, "w2": w2_b,
        "bqp": part_tiles(bq, 8, S_A), "bkp": part_tiles(bk, 8, S_A),
        "bvap": bva_b, "bor": bo_r,
        "b1p": part_tiles(b1, 32), "b2p": part_tiles(b2, 8),
        "g1p": part_tiles(ln1_g, 8, S_A), "bb1p": part_tiles(ln1_b, 8, S_A),
        "g3p": part_tiles(ln3_g, 8, S_A), "bb3p": part_tiles(ln3_b, 8, S_A),
    }
    in_maps = []
    for c in range(N_CORES):
        b, hh = c // 2, c % 2
        rows = tgt[b, hh * R:(hh + 1) * R]            # [512, 1024]
        m = {"tgtT": np.ascontiguousarray(rows.T),
             "memT": np.ascontiguousarray(to8(memory[b].T, S_A))}
        m.update(shared)
        in_maps.append(m)
    return in_maps


def kernel(tgt, memory, ln1_g, ln1_b, wq, bq, wk, bk, wv, bv, wo, bo,
           ln3_g, ln3_b, w1, b1, w2, b2):
    in_maps = _prep_inputs(tgt, memory, ln1_g, ln1_b, wq, bq, wk, bk, wv, bv,
                           wo, bo, ln3_g, ln3_b, w1, b1, w2, b2)
    nc = _get_program()
    res = run_bass_kernel_spmd(nc, in_maps, list(range(N_CORES)))

    out = np.empty((B, LQ, E), np.float32)
    attn = np.empty((B, LQ, LK), np.float32)
    for c in range(N_CORES):
        b, hh = c // 2, c % 2
        out[b, hh * R:(hh + 1) * R] = res.results[c]["outT"].T
        attn[b, hh * R:(hh + 1) * R] = np.asarray(res.results[c]["attnT"],
                                                  np.float32).T
    return out, attn
